# revision 12
# baseline (speedup 1.0000x reference)
"""T5-style encoder self-attention (dense_transformer) on 8 Trainium2 NeuronCores.

Problem (full shapes): hidden [2,2048,2048], Wq/Wk/Wv/Wo [2048,2048],
rel_emb [32,32] (bidirectional T5 relative-position bias), mask [2,1,1,2048].

Sharding: data-parallel over batch (2) x tensor-parallel over heads (4 groups
of 8 heads) = 8 cores, Megatron-style. Each core computes a partial output
[2048,2048] for its batch (its 8 heads through its Wo row-slice); the host
sums 4 partials per batch.

Per-core kernel design (bf16 operands, fp32 PSUM accumulation):
  - The relative-position bias is applied MULTIPLICATIVELY for every tile:
    px = exp(s/8 + mask) * erel, where erel = exp(bias) is a host-computed
    [8 heads, 4096 diagonals] bf16 table read through a Toeplitz shear view
    (partition stride 1, free stride 1).  Host numerics sim puts the
    all-multiplicative absmax-rel at 7.7e-3 (vs 6.0e-3 for the old scheme
    that identity-injected near-diagonal tiles into PSUM; the inject matmuls
    cost ~38us of PE issue time and a 16us identity DMA on the critical
    startup path).
  - Phase B: single pass over x^T computes pair-0 Q^T/K^T and V for ALL
    heads (6 matmuls per x^T chunk, PE-bound).  Q^T is stored with s
    REVERSED so the bias becomes a positive-shear Toeplitz.
  - Phase C attention, per (head-pair, q-chunk), k-tile loop pipelined one
    iteration ahead:
      * the two per-head QK matmuls are packed as concurrent 64-row-group
        tiles (tile_position (0,0)/(64,0));
      * ACT computes exp(s/8 + mask) in one [128,1024] shot per k-tile;
        DVE multiplies by the erel shear slice (far AND near tiles);
      * next-pair Q/K projection matmuls are interleaved PER k-tile so they
        fill the PE's ACT-wait bubbles; their x^T tiles are group-loaded
        (4 k-chunks per DMA, 4KB partition lines) and prefetched one group
        ahead so the proj matmuls never wait on DMA.
  - x^T is host-tiled to [128, NQC, NDT, 512] so every [128, 4, 512] group
    load has 4KB contiguous per-partition lines (the old [D,S] layout gave
    1KB lines, which capped each DMA queue near 85 GB/s and stalled the
    interleaved projections).
  - V augmentation: per pair, even head block = [v(0:64) | ones(64)] (M=65,
    denominator lands on PSUM partition 64), odd head block = 128 wide with
    ones at col 32 and v at cols 64:128 (denominator on partition 32, ctx on
    partitions 64:128), keeping every normalize op partition-aligned.
  - Normalize is DEFERRED and PE-free: cx evacuates to SBUF at qc end
    (freeing its PSUM slot), then one qc later a DVE+DMA-only chain runs:
    pack denominator rows to a base-0 tile (custom DVE ops require base
    partition 0), reciprocal_approx_fast, bounce the two reciprocal rows
    through DRAM, stride-0 DMAs broadcast them across partitions, and fused
    DVE tensor_tensors do normalize + un-reverse + bf16 writeback.
  - The next qc's first score-group is pre-emitted in the current qc's tail
    (exactly one PSUM slot is free there) so ACT never idles at boundaries.
  - Startup: the first x^T group and the first wq/wk/wv chunk are the FIRST
    DMAs on their queues (sync / gpsimd), so the first matmul fires ~9us in
    instead of ~41us; mask + ACT-table warmup + wo ride the scalar queue.
  - Phase D output projection: descending s-tiles (low tiles depend on the
    last deferred normalize), m looped inside nd so consecutive matmuls hit
    different PSUM banks; evacuation alternates ACT/DVE; the two out DMAs
    per s-tile alternate sync/gpsimd queues.
"""

import math
import sys

for _p in ("/opt/trn_rl_repo",):
    if _p not in sys.path:
        sys.path.insert(0, _p)

import numpy as np

import concourse.bass as bass
import concourse.mybir as mybir
import concourse.tile as tile
from concourse import bacc
from concourse.bass_utils import run_bass_kernel_spmd

DT = mybir.dt
AF = mybir.ActivationFunctionType
OP = mybir.AluOpType

# ---- problem constants (hardcoded per contract) ----
B, S, D = 2, 2048, 2048
N_HEADS, D_KV = 32, 64
NUM_BUCKETS, MAX_DISTANCE = 32, 128
NCORES = 8
HL = 8            # heads per core
P = 128
SC = 512          # free-dim chunk
NKT = S // P      # 16 k-tiles
NQC = S // SC     # 4 q-chunks
NDT = D // P      # 16 D-tiles
NMT = (HL * D_KV) // P   # 4 hd m-tiles per core
NPAIR = HL // 2   # 4 head pairs per core
NDIAG = 4096
W_U = 3968        # erel shear tile width (covers all diagonals any tile hits)
VW = 193          # vaug per-(kt,pair) width: even block 65 + odd block 128
NKG = NDT // 4    # 4 kd-groups of 4 chunks per q-chunk (x^T group loads)


def _rel_bucket_host(d):
    """Exact numpy replica of reference._relative_position_bucket."""
    num_buckets = NUM_BUCKETS // 2          # 16
    max_exact = num_buckets // 2            # 8
    rel = np.asarray(d, dtype=np.int64)
    buckets = (rel > 0).astype(np.int32) * num_buckets
    arel = np.abs(rel)
    is_small = arel < max_exact
    rp_safe = np.maximum(arel, 1).astype(np.float32)
    log_ratio = np.log(rp_safe / np.float32(max_exact)).astype(np.float32)
    scale = np.float32(math.log(MAX_DISTANCE / max_exact))
    rp_large = max_exact + (log_ratio / scale * np.float32(num_buckets - max_exact)).astype(np.int32)
    rp_large = np.minimum(rp_large, num_buckets - 1)
    buckets = buckets + np.where(is_small, arel.astype(np.int32), rp_large)
    return buckets.astype(np.int32)


def _bias_table(rel_emb_slice):
    """rel_emb_slice: [NUM_BUCKETS, HL] fp32 -> erel [HL, NDIAG] bf16,
    erel[h, i] = exp(bias(d = i - 2047)); erel[:, 4095] is never read."""
    import ml_dtypes
    i = np.arange(NDIAG - 1)
    b = _rel_bucket_host(i - (S - 1))                  # [4095]
    vals = rel_emb_slice[b, :]                         # [4095, HL] fp32
    erel = np.zeros((HL, NDIAG), dtype=np.float32)
    erel[:, : NDIAG - 1] = np.exp(vals.T)
    return erel.astype(ml_dtypes.bfloat16)


def _build():
    nc = bacc.Bacc(None, name="attn_tp")

    # x^T host-tiled: xt[p, qc, kd, j] = x[qc*512+j, kd*128+p], so a
    # [128, 4, 512] kd-group load is one DMA with 4KB per-partition lines
    xt = nc.declare_dram_parameter("xt", [P, NQC, NDT, SC], DT.bfloat16,
                                   isOutput=False)
    # weights arrive HOST-SHUFFLED to [p][kt][h] so per-partition lines are
    # contiguous multi-KB runs (DMA packet rate is the limiter at 1KB lines)
    wq = nc.declare_dram_parameter("wq", [P, NDT * HL * D_KV], DT.bfloat16, isOutput=False)
    wk = nc.declare_dram_parameter("wk", [P, NDT * HL * D_KV], DT.bfloat16, isOutput=False)
    wv = nc.declare_dram_parameter("wv", [P, NDT * HL * D_KV], DT.bfloat16, isOutput=False)
    wo = nc.declare_dram_parameter("wo", [P, NMT * D], DT.bfloat16, isOutput=False)
    mask = nc.declare_dram_parameter("mask", [S], DT.float32, isOutput=False)
    erel = nc.declare_dram_parameter("erel", [HL, NDIAG], DT.bfloat16, isOutput=False)
    out = nc.declare_dram_parameter("out", [S, D], DT.float32, isOutput=True)

    with tile.TileContext(nc) as tc:
        with (
            tc.tile_pool(name="res", bufs=1) as res,          # persistent tensors
            tc.tile_pool(name="xtp", bufs=3) as xtp,          # x^T groups (sync q)
            tc.tile_pool(name="upool", bufs=2) as upool,      # exp-bias shear tiles
            tc.tile_pool(name="pexp", bufs=3) as pexpp,       # probs tiles
            tc.tile_pool(name="stage", bufs=2) as stage,      # normalize staging
            tc.tile_pool(name="outp", bufs=3) as outp,        # out staging
            tc.tile_pool(name="psum", bufs=4, space="PSUM") as psum,  # [128,1024] slots
            tc.tile_pool(name="dram", bufs=2, space="DRAM") as dramp,
        ):
            # ---------- constants / resident tensors ----------
            mask_sb = res.tile([P, NKT], DT.float32, tag="mask")
            # mask + ACT exp-table warm-up ride the (otherwise idle) scalar
            # queue so the sync/gpsimd queues start with the critical loads
            nc.scalar.dma_start(mask_sb[:], mask.ap().rearrange("(kt p) -> p kt", p=P))

            wq_sb = res.tile([P, NDT, HL * D_KV], DT.bfloat16, tag="wq")
            wk_sb = res.tile([P, NDT, HL * D_KV], DT.bfloat16, tag="wk")
            wv_sb = res.tile([P, NDT, HL * D_KV], DT.bfloat16, tag="wv")
            wo_sb = res.tile([P, NMT, D], DT.bfloat16, tag="wo")

            # persistent activations.  qt/kt/ctxt are split PER PAIR so the
            # tile dep tracker never serializes pair pr's score reads behind
            # pair pr+1's projection-drain writes (false WAR at qc bounds).
            qt_p = [res.tile([P, S], DT.bfloat16, tag=f"qt{m}", name=f"qt{m}")
                    for m in range(NMT)]                       # q REVERSED
            kt_p = [res.tile([P, S], DT.bfloat16, tag=f"kt{m}", name=f"kt{m}")
                    for m in range(NMT)]
            vaug = res.tile([P, NKT, NPAIR, VW], DT.bfloat16, tag="vaug")
            ctxt_p = [res.tile([P, S], DT.bfloat16, tag=f"ctxt{m}", name=f"ctxt{m}")
                      for m in range(NMT)]
            # only the two ones-columns are ever read outside the V blocks
            # (psum rows other than the denominator rows are never consumed)
            nc.vector.memset(vaug[:, :, :, 64:65], 1.0)
            nc.vector.memset(vaug[:, :, :, 97:98], 1.0)

            # ACT exp table warm-up (hide the ~2.7us table load under phase B)
            warm = res.tile([1, 2], DT.float32, tag="warm")
            nc.scalar.activation(out=warm[0:1, 0:1], in_=mask_sb[0:1, 0:1], func=AF.Exp)

            def rev_ap(base, jg0):
                """reversed-q view: base is a [rows, S] AP slice of a res
                tensor; returns [rows, SC] AP walking q backwards so writing
                reversed data lands in natural order."""
                return bass.AP(
                    tensor=base.tensor,
                    offset=base.offset + (S - 1 - jg0),
                    ap=[list(base.ap[0]), [-1, SC]],
                )

            UQ = W_U // 4
            def load_u(pr, quarter=None, u=None, eng=None):
                """erel shear tile [P, 2, W_U] for pair pr: u[p, i, w] =
                erel[2*pr+i, p + w].  quarter=None loads everything;
                otherwise loads one quarter of each head's span into the
                passed tile (spreads the ~2 MB burst across the previous
                pair's four q-chunks)."""
                if u is None:
                    u = upool.tile([P, 2, W_U], DT.bfloat16, tag="u",
                                   name=f"u{pr}", bufs=2)
                ap0 = erel.ap()
                qs = range(4) if quarter is None else [quarter]
                for i, hh in enumerate((2 * pr, 2 * pr + 1)):
                    for qq in qs:
                        shear = bass.AP(
                            tensor=ap0.tensor,
                            offset=ap0.offset + hh * NDIAG + qq * UQ,
                            ap=[[1, P], [1, UQ]],
                        )
                        (eng or nc.sync).dma_start(
                            u[:, i, qq * UQ:(qq + 1) * UQ], shear)
                return u

            def load_wchunk(g):
                """one 4-kd chunk of weights: wq/wk gpsimd, wv scalar."""
                cw = HL * D_KV
                c0, c1 = g * 4 * cw, (g + 1) * 4 * cw
                nc.gpsimd.dma_start(wq_sb[:, g * 4:(g + 1) * 4, :], wq[:, c0:c1])
                nc.gpsimd.dma_start(wk_sb[:, g * 4:(g + 1) * 4, :], wk[:, c0:c1])
                nc.scalar.dma_start(wv_sb[:, g * 4:(g + 1) * 4, :], wv[:, c0:c1])

            def load_xgroup(nq, g):
                """one [128, 4, 512] x^T kd-group (4KB partition lines)."""
                t = xtp.tile([P, 4, SC], DT.bfloat16, tag="xt",
                             name=f"xg{nq}_{g}")
                nc.sync.dma_start(t[:], xt[:, nq, 4 * g:4 * (g + 1), :])
                return t

            # ---------- phase B: pair-0 Q/K + V (all heads), single x^T pass ----
            # first weight chunk + first x group lead their queues so the
            # first matmul fires as soon as ~1MB lands (~9us)
            load_wchunk(0)
            for nq in range(NQC):
                qk_ps = psum.tile([P, 2 * SC], DT.float32, tag="ps",
                                  name=f"qkps0_{nq}")
                q_ps, k_ps = qk_ps[:, 0:SC], qk_ps[:, SC:2 * SC]
                v01 = psum.tile([P, 2 * SC], DT.float32, tag="ps", name=f"v01_{nq}")
                v23 = psum.tile([P, 2 * SC], DT.float32, tag="ps", name=f"v23_{nq}")
                v_ps = [v01[:, 0:SC], v01[:, SC:2 * SC],
                        v23[:, 0:SC], v23[:, SC:2 * SC]]
                for g in range(NKG):
                    if nq == 0 and g + 1 < NKG:
                        load_wchunk(g + 1)   # prefetch next weight chunk
                    xg = load_xgroup(nq, g)
                    for c in range(4):
                        kd = 4 * g + c
                        xt_t = xg[:, c, :]
                        nc.tensor.matmul(
                            q_ps, wq_sb[:, kd, 0:P], xt_t,
                            start=(kd == 0), stop=(kd == NDT - 1),
                        )
                        nc.tensor.matmul(
                            k_ps, wk_sb[:, kd, 0:P], xt_t,
                            start=(kd == 0), stop=(kd == NDT - 1),
                        )
                        for st in range(4):
                            nc.tensor.matmul(
                                v_ps[st], xg[:, c, st * P:(st + 1) * P],
                                wv_sb[:, kd, :],
                                start=(kd == 0), stop=(kd == NDT - 1),
                            )
                if nq == 0:
                    nc.scalar.dma_start(
                        wo_sb.rearrange("p a b -> p (a b)"), wo[:])
                # pair-0 u table: one quarter per nq, off the sync queue
                u0 = load_u(0, quarter=nq, u=None if nq == 0 else u0)
                # drain: V -> vaug blocks first (frees the 2 V psum slots the
                # next nq's V matmuls are waiting on), then q/k casts
                for st in range(4):
                    ktg = nq * 4 + st
                    vsrc = v_ps[st].rearrange("p (pr par d) -> p pr par d",
                                              par=2, d=D_KV)
                    nc.vector.tensor_copy(vaug[:, ktg, :, 0:D_KV],
                                          vsrc[:, :, 0, :])
                    nc.vector.tensor_copy(vaug[:, ktg, :, 129:193],
                                          vsrc[:, :, 1, :])
                nc.vector.tensor_copy(rev_ap(qt_p[0][:, :], nq * SC), q_ps)
                nc.vector.tensor_copy(kt_p[0][:, nq * SC:(nq + 1) * SC], k_ps)

            # ---------- phase C: attention, proj of pair pr+1 interleaved ----
            def emit_sg(pr, qc, kt):
                """scores psum group for (pair, q-chunk, k-tile): the two
                heads run as concurrent 64-row-group tiles."""
                jg0 = qc * SC
                s01 = psum.tile([P, 2 * SC], DT.float32, tag="ps",
                                name=f"s{pr}_{qc}_{kt}")
                nc.tensor.matmul(
                    s01[:, 0:SC], kt_p[pr][0:64, kt * P:(kt + 1) * P],
                    qt_p[pr][0:64, jg0:jg0 + SC],
                    start=True, stop=True, tile_position=(0, 0),
                )
                nc.tensor.matmul(
                    s01[:, SC:2 * SC], kt_p[pr][64:128, kt * P:(kt + 1) * P],
                    qt_p[pr][64:128, jg0:jg0 + SC],
                    start=True, stop=True, tile_position=(64, 0),
                )
                return s01

            # proj x^T group tiles, prefetched one group ahead (keyed by
            # group index within the current (proj, qc))
            def load_pgroup(proj, qc, g):
                t = xtp.tile([P, 4, SC], DT.bfloat16, tag="xt",
                             name=f"xp{proj}_{qc}_{g}")
                nc.sync.dma_start(t[:], xt[:, qc, 4 * g:4 * (g + 1), :])
                return t

            def attn_qc(pr, qc, u_t, proj, pending, s_pre, nxt_sg, pg0):
                """attention for head pair pr, reversed-q chunk qc.
                proj: None or pr+1 (emit that pair's Q/K proj, 1 kd per kt).
                pg0: pre-loaded x^T group 0 for the proj (or None).
                Returns (normalize closure, pre-emitted next score group,
                pre-loaded group 0 for the NEXT (proj, qc))."""
                h0, h1 = 2 * pr, 2 * pr + 1
                jg0 = qc * SC
                cx01 = psum.tile([P, 2 * SC], DT.float32, tag="ps",
                                 name=f"cx{pr}_{qc}")
                if proj is not None:
                    pj_ps = psum.tile([P, 2 * SC], DT.float32, tag="ps",
                                      name=f"pjps{proj}_{qc}")
                    pjq, pjk = pj_ps[:, 0:SC], pj_ps[:, SC:2 * SC]
                    pgs = {0: pg0}

                def emit_proj(kt):
                    g, c = kt // 4, kt % 4
                    if c == 0 and g + 1 < NKG:
                        pgs[g + 1] = load_pgroup(proj, qc, g + 1)
                    kd = kt
                    xt_t = pgs[g][:, c, :]
                    nc.tensor.matmul(
                        pjq, wq_sb[:, kd, proj * P:(proj + 1) * P], xt_t,
                        start=(kd == 0), stop=(kd == NDT - 1),
                    )
                    nc.tensor.matmul(
                        pjk, wk_sb[:, kd, proj * P:(proj + 1) * P], xt_t,
                        start=(kd == 0), stop=(kd == NDT - 1),
                    )

                # 2-deep software pipeline: s(kt+2) is emitted before PV(kt)
                # so the in-order PE queue keeps a backlog (hides LDWEIGHTS
                # and cross-engine semaphore latency).  pending() emits the
                # PREVIOUS qc's deferred normalize chain (DVE+DMA only).
                sq = [s_pre if s_pre is not None else emit_sg(pr, qc, 0),
                      emit_sg(pr, qc, 1)]
                for kt in range(NKT):
                    if kt + 2 < NKT:
                        sq.append(emit_sg(pr, qc, kt + 2))
                    if proj is not None:
                        emit_proj(kt)
                    if kt == 2 and pending is not None:
                        pending()
                    s01 = sq[kt]
                    px = pexpp.tile([P, 2 * SC], DT.bfloat16, tag="pexp",
                                    name=f"px{pr}_{qc}_{kt}")
                    nc.scalar.activation(
                        out=px[:], in_=s01[:], func=AF.Exp,
                        bias=mask_sb[:, kt:kt + 1], scale=1.0 / math.sqrt(D_KV),
                    )
                    j0 = kt * P + jg0
                    nc.vector.tensor_tensor(
                        px.rearrange("p (h j) -> p h j", h=2),
                        px.rearrange("p (h j) -> p h j", h=2),
                        u_t[:, :, j0:j0 + SC], OP.mult
                    )
                    nc.tensor.matmul(
                        cx01[0:65, 0:SC], vaug[:, kt, pr, 0:65], px[:, 0:SC],
                        start=(kt == 0), stop=(kt == NKT - 1),
                    )
                    nc.tensor.matmul(
                        cx01[:, SC:2 * SC], vaug[:, kt, pr, 65:VW],
                        px[:, SC:2 * SC],
                        start=(kt == 0), stop=(kt == NKT - 1),
                    )

                # proj drain (reversed q for qt)
                if proj is not None:
                    nc.scalar.copy(rev_ap(qt_p[proj][:, :], jg0), pjq)
                    nc.vector.tensor_copy(
                        kt_p[proj][:, jg0:jg0 + SC], pjk)

                # prefetch group 0 of the NEXT (proj, qc)'s x^T
                pg_next = None
                if qc + 1 < NQC and proj is not None:
                    pg_next = load_pgroup(proj, qc + 1, 0)
                elif qc == NQC - 1 and proj is not None and proj + 1 < NPAIR:
                    pg_next = load_pgroup(proj + 1, 0, 0)

                # pre-emit the NEXT qc's first score group so ACT never idles
                # across the boundary (exactly one PSUM slot is free here)
                s_next = nxt_sg() if nxt_sg is not None else None

                # ---- evacuate cx to SBUF (frees the PSUM slot), then the
                # rest of normalize+writeback is DEFERRED into the next qc
                # (DVE + DMA only; the PE never touches it) ----
                cxs = stage.tile([P, 2 * SC], DT.bfloat16, tag="cxs",
                                 name=f"cxs{pr}_{qc}", bufs=1)
                nc.vector.tensor_copy(cxs[:], cx01[:])

                def normalize():
                    # denominators: h0 on row 64 (cols 0:512), h1 on row 32
                    # (cols 512:1024).  Custom DVE ops need base-partition-0
                    # operands, so pack both rows into a base-0 tile first.
                    dnf = stage.tile([P, SC], DT.float32, tag="dnf",
                                     name=f"dnf{pr}_{qc}", bufs=1)
                    nc.vector.tensor_copy(dnf[64:65, :], cxs[64:65, 0:SC])
                    nc.vector.tensor_copy(dnf[32:33, :], cxs[32:33, SC:2 * SC])
                    rb = stage.tile([P, SC], DT.float32, tag="rb",
                                    name=f"rb{pr}_{qc}", bufs=1)
                    nc.vector.reciprocal_approx_fast(out=rb[:], in_=dnf[:])
                    rbh = stage.tile([P, SC], DT.bfloat16, tag="rbh",
                                     name=f"rbh{pr}_{qc}", bufs=1)
                    nc.vector.tensor_copy(rbh[64:65, :], rb[64:65, :])
                    nc.vector.tensor_copy(rbh[32:33, :], rb[32:33, :])
                    # broadcast across partitions: bounce the two reciprocal
                    # rows through DRAM, then stride-0 DMA reads replicate
                    # them to 64 partitions each (all off the engine queues).
                    bnc = dramp.tile([2, SC], DT.bfloat16, tag="bnc",
                                     name=f"bnc{pr}_{qc}")
                    nc.gpsimd.dma_start(bnc[0:1, :], rbh[64:65, :])
                    nc.gpsimd.dma_start(bnc[1:2, :], rbh[32:33, :])
                    bc_sb = stage.tile([P, SC], DT.bfloat16, tag="bc",
                                       name=f"bcs{pr}_{qc}", bufs=1)
                    src0 = bass.AP(tensor=bnc.tensor, offset=bnc.offset,
                                   ap=[[0, 64], [1, SC]])
                    src1 = bass.AP(tensor=bnc.tensor, offset=bnc.offset + SC,
                                   ap=[[0, 64], [1, SC]])
                    nc.gpsimd.dma_start(bc_sb[0:64, :], src0)
                    nc.gpsimd.dma_start(bc_sb[64:128, :], src1)
                    nc.vector.tensor_tensor(
                        rev_ap(ctxt_p[pr][0:64, :], jg0),
                        cxs[0:64, 0:SC], bc_sb[0:64, :], OP.mult)
                    nc.vector.tensor_tensor(
                        rev_ap(ctxt_p[pr][64:128, :], jg0),
                        cxs[64:128, SC:2 * SC], bc_sb[64:128, :], OP.mult)
                return normalize, s_next, pg_next

            u_t = u0  # pair-0 table already quarter-loaded during phase B
            pending = None
            s_pre = None
            pg0 = load_pgroup(1, 0, 0)
            seq = [(pr, qc) for pr in range(NPAIR) for qc in range(NQC)]
            for idx, (pr, qc) in enumerate(seq):
                nxt = pr + 1 if pr + 1 < NPAIR else None
                if nxt is not None:
                    next_u = load_u(nxt, quarter=qc,
                                    u=None if qc == 0 else next_u,
                                    eng=nc.gpsimd)
                if idx + 1 < len(seq):
                    npr, nqc = seq[idx + 1]
                    nxt_sg = (lambda npr=npr, nqc=nqc: emit_sg(npr, nqc, 0))
                else:
                    nxt_sg = None
                pending, s_pre, pg0 = attn_qc(pr, qc, u_t, nxt, pending,
                                              s_pre, nxt_sg, pg0)
                if qc == NQC - 1 and nxt is not None:
                    u_t = next_u
            pending()

            # ---------- phase D: output projection (descending st: the
            # low-st tiles depend on the last deferred normalize) ----------
            for st in range(NKT - 1, -1, -1):
                oa = psum.tile([P, 2 * SC], DT.float32, tag="ps",
                               name=f"oa{st}")
                ob = psum.tile([P, 2 * SC], DT.float32, tag="ps",
                               name=f"ob{st}")
                o_ps = [oa[:, 0:SC], oa[:, SC:2 * SC],
                        ob[:, 0:SC], ob[:, SC:2 * SC]]
                for m in range(NMT):
                    for nd in range(NQC):
                        nc.tensor.matmul(
                            o_ps[nd], ctxt_p[m][:, st * P:(st + 1) * P],
                            wo_sb[:, m, nd * SC:(nd + 1) * SC],
                            start=(m == 0), stop=(m == NMT - 1),
                        )
                for half in range(2):
                    o_t = outp.tile([P, 2, SC], DT.float32, tag="out",
                                    name=f"ot{st}_{half}")
                    nc.scalar.copy(o_t[:, 0, :], o_ps[2 * half])
                    nc.vector.tensor_copy(o_t[:, 1, :], o_ps[2 * half + 1])
                    eng = nc.sync if half == 0 else nc.gpsimd
                    eng.dma_start(
                        out[st * P:(st + 1) * P,
                            half * 2 * SC:(half + 1) * 2 * SC],
                        o_t[:],
                    )

    nc.finalize()
    return nc


_NC_CACHE = None


def _get_nc():
    global _NC_CACHE
    if _NC_CACHE is None:
        _NC_CACHE = _build()
    return _NC_CACHE


def _in_maps(hidden_states, attention_mask, Wq, Wk, Wv, Wo, rel_emb):
    import ml_dtypes
    bf16 = ml_dtypes.bfloat16
    maps = []
    for c in range(NCORES):
        b, g = c // 4, c % 4
        hlo, hhi = g * HL, (g + 1) * HL
        erel = _bias_table(
            np.ascontiguousarray(rel_emb[:, hlo:hhi], dtype=np.float32))
        def shuf(w):  # [NDT*P, C] -> [P, NDT*C] partition-contiguous
            cc = w.shape[1]
            return np.ascontiguousarray(
                w.reshape(-1, P, cc).transpose(1, 0, 2).reshape(P, -1))
        # xt[p, qc, kd, j] = x[qc*512+j, kd*128+p]
        xtt = np.ascontiguousarray(
            hidden_states[b].reshape(NQC, SC, NDT, P).transpose(3, 0, 2, 1)
        ).astype(bf16)
        maps.append({
            "xt": xtt,
            "wq": shuf(Wq[:, hlo * D_KV:hhi * D_KV]).astype(bf16),
            "wk": shuf(Wk[:, hlo * D_KV:hhi * D_KV]).astype(bf16),
            "wv": shuf(Wv[:, hlo * D_KV:hhi * D_KV]).astype(bf16),
            "wo": shuf(Wo[hlo * D_KV:hhi * D_KV, :]).astype(bf16),
            "mask": np.ascontiguousarray(attention_mask[b, 0, 0, :]).astype(np.float32),
            "erel": erel,
        })
    return maps


def kernel(hidden_states, attention_mask, Wq, Wk, Wv, Wo, rel_emb, _trace=False,
           _trace_kwargs=None):
    hidden_states = np.asarray(hidden_states, dtype=np.float32)
    attention_mask = np.asarray(attention_mask, dtype=np.float32)
    Wq = np.asarray(Wq, dtype=np.float32)
    Wk = np.asarray(Wk, dtype=np.float32)
    Wv = np.asarray(Wv, dtype=np.float32)
    Wo = np.asarray(Wo, dtype=np.float32)
    rel_emb = np.asarray(rel_emb, dtype=np.float32)

    nc = _get_nc()
    maps = _in_maps(hidden_states, attention_mask, Wq, Wk, Wv, Wo, rel_emb)
    kw = dict(_trace_kwargs or {})
    res = run_bass_kernel_spmd(nc, maps, core_ids=list(range(NCORES)),
                               trace=_trace, **kw)
    kernel.last_results = res
    outp = np.empty((B, S, D), dtype=np.float32)
    for b in range(B):
        acc = np.asarray(res.results[4 * b]["out"], dtype=np.float32).copy()
        for g in range(1, 4):
            acc += np.asarray(res.results[4 * b + g]["out"], dtype=np.float32)
        outp[b] = acc
    return outp


# revision 13
# speedup vs baseline: 1.0496x; 1.0496x over previous
"""T5-style encoder self-attention (dense_transformer) on 8 Trainium2 NeuronCores.

Problem (full shapes): hidden [2,2048,2048], Wq/Wk/Wv/Wo [2048,2048],
rel_emb [32,32] (bidirectional T5 relative-position bias), mask [2,1,1,2048].

Sharding: data-parallel over batch (2) x tensor-parallel over heads (4 groups
of 8 heads) = 8 cores, Megatron-style. Each core computes a partial output
[2048,2048] for its batch (its 8 heads through its Wo row-slice); the host
sums 4 partials per batch.

Per-core kernel design (bf16 operands, fp32 PSUM accumulation):
  - The relative-position bias is applied MULTIPLICATIVELY for every tile:
    px = exp(s/8 + mask) * erel, where erel = exp(bias) is a host-computed
    [8 heads, 4096 diagonals] bf16 table read through a Toeplitz shear view
    (partition stride 1, free stride 1).  Host numerics sim puts the
    all-multiplicative absmax-rel at 7.7e-3 (vs 6.0e-3 for the old scheme
    that identity-injected near-diagonal tiles into PSUM; the inject matmuls
    cost ~38us of PE issue time and a 16us identity DMA on the critical
    startup path).
  - Phase B: single pass over x^T computes pair-0 Q^T/K^T and V for ALL
    heads (6 matmuls per x^T chunk, PE-bound).  Q^T is stored with s
    REVERSED so the bias becomes a positive-shear Toeplitz.
  - Phase C attention, per (head-pair, q-chunk), k-tile loop pipelined one
    iteration ahead:
      * the two per-head QK matmuls are packed as concurrent 64-row-group
        tiles (tile_position (0,0)/(64,0));
      * ACT computes exp(s/8 + mask) in one [128,1024] shot per k-tile;
        DVE multiplies by the erel shear slice (far AND near tiles);
      * next-pair Q/K projection matmuls are interleaved PER k-tile so they
        fill the PE's ACT-wait bubbles; their x^T tiles are group-loaded
        (4 k-chunks per DMA, 4KB partition lines) and prefetched one group
        ahead so the proj matmuls never wait on DMA.
  - x^T is host-tiled to [128, NQC, NDT, 512] so every [128, 4, 512] group
    load has 4KB contiguous per-partition lines (the old [D,S] layout gave
    1KB lines, which capped each DMA queue near 85 GB/s and stalled the
    interleaved projections).
  - V augmentation: per pair, even head block = [v(0:64) | ones(64)] (M=65,
    denominator lands on PSUM partition 64), odd head block = 128 wide with
    ones at col 32 and v at cols 64:128 (denominator on partition 32, ctx on
    partitions 64:128), keeping every normalize op partition-aligned.
  - Normalize is DEFERRED and PE-free: cx evacuates to SBUF at qc end
    (freeing its PSUM slot), then one qc later a DVE+DMA-only chain runs:
    pack denominator rows to a base-0 tile (custom DVE ops require base
    partition 0), reciprocal_approx_fast, bounce the two reciprocal rows
    through DRAM, stride-0 DMAs broadcast them across partitions, and fused
    DVE tensor_tensors do normalize + un-reverse + bf16 writeback.
  - The next qc's first score-group is pre-emitted in the current qc's tail
    (exactly one PSUM slot is free there) so ACT never idles at boundaries.
  - Startup: the first x^T group and the first wq/wk/wv chunk are the FIRST
    DMAs on their queues (sync / gpsimd), so the first matmul fires ~9us in
    instead of ~41us; mask + ACT-table warmup + wo ride the scalar queue.
  - Phase D output projection: descending s-tiles (low tiles depend on the
    last deferred normalize), m looped inside nd so consecutive matmuls hit
    different PSUM banks; evacuation alternates ACT/DVE; the two out DMAs
    per s-tile alternate sync/gpsimd queues.
"""

import math
import sys

for _p in ("/opt/trn_rl_repo",):
    if _p not in sys.path:
        sys.path.insert(0, _p)

import numpy as np

import concourse.bass as bass
import concourse.mybir as mybir
import concourse.tile as tile
from concourse import bacc
from concourse.bass_utils import run_bass_kernel_spmd

DT = mybir.dt
AF = mybir.ActivationFunctionType
OP = mybir.AluOpType

# ---- problem constants (hardcoded per contract) ----
B, S, D = 2, 2048, 2048
N_HEADS, D_KV = 32, 64
NUM_BUCKETS, MAX_DISTANCE = 32, 128
NCORES = 8
HL = 8            # heads per core
P = 128
SC = 512          # free-dim chunk
NKT = S // P      # 16 k-tiles
NQC = S // SC     # 4 q-chunks
NDT = D // P      # 16 D-tiles
NMT = (HL * D_KV) // P   # 4 hd m-tiles per core
NPAIR = HL // 2   # 4 head pairs per core
NDIAG = 4096
W_U = 3968        # erel shear tile width (covers all diagonals any tile hits)
VW = 193          # vaug per-(kt,pair) width: even block 65 + odd block 128
NKG = NDT // 4    # 4 kd-groups of 4 chunks per q-chunk (x^T group loads)


def _rel_bucket_host(d):
    """Exact numpy replica of reference._relative_position_bucket."""
    num_buckets = NUM_BUCKETS // 2          # 16
    max_exact = num_buckets // 2            # 8
    rel = np.asarray(d, dtype=np.int64)
    buckets = (rel > 0).astype(np.int32) * num_buckets
    arel = np.abs(rel)
    is_small = arel < max_exact
    rp_safe = np.maximum(arel, 1).astype(np.float32)
    log_ratio = np.log(rp_safe / np.float32(max_exact)).astype(np.float32)
    scale = np.float32(math.log(MAX_DISTANCE / max_exact))
    rp_large = max_exact + (log_ratio / scale * np.float32(num_buckets - max_exact)).astype(np.int32)
    rp_large = np.minimum(rp_large, num_buckets - 1)
    buckets = buckets + np.where(is_small, arel.astype(np.int32), rp_large)
    return buckets.astype(np.int32)


def _bias_table(rel_emb_slice):
    """rel_emb_slice: [NUM_BUCKETS, HL] fp32 -> erel [HL, NDIAG] bf16,
    erel[h, i] = exp(bias(d = i - 2047)); erel[:, 4095] is never read."""
    import ml_dtypes
    i = np.arange(NDIAG - 1)
    b = _rel_bucket_host(i - (S - 1))                  # [4095]
    vals = rel_emb_slice[b, :]                         # [4095, HL] fp32
    erel = np.zeros((HL, NDIAG), dtype=np.float32)
    erel[:, : NDIAG - 1] = np.exp(vals.T)
    return erel.astype(ml_dtypes.bfloat16)


def _build():
    nc = bacc.Bacc(None, name="attn_tp")

    # x^T host-tiled: xt[p, qc, kd, j] = x[qc*512+j, kd*128+p], so a
    # [128, 4, 512] kd-group load is one DMA with 4KB per-partition lines
    xt = nc.declare_dram_parameter("xt", [P, NQC, NDT, SC], DT.bfloat16,
                                   isOutput=False)
    # weights arrive HOST-SHUFFLED to [p][kt][h] so per-partition lines are
    # contiguous multi-KB runs (DMA packet rate is the limiter at 1KB lines)
    wq = nc.declare_dram_parameter("wq", [P, NDT * HL * D_KV], DT.bfloat16, isOutput=False)
    wk = nc.declare_dram_parameter("wk", [P, NDT * HL * D_KV], DT.bfloat16, isOutput=False)
    wv = nc.declare_dram_parameter("wv", [P, NDT * HL * D_KV], DT.bfloat16, isOutput=False)
    wo = nc.declare_dram_parameter("wo", [P, NMT * D], DT.bfloat16, isOutput=False)
    mask = nc.declare_dram_parameter("mask", [S], DT.float32, isOutput=False)
    erel = nc.declare_dram_parameter("erel", [HL, NDIAG], DT.bfloat16, isOutput=False)
    out = nc.declare_dram_parameter("out", [S, D], DT.float32, isOutput=True)

    with tile.TileContext(nc) as tc:
        with (
            tc.tile_pool(name="res", bufs=1) as res,          # persistent tensors
            tc.tile_pool(name="xtp", bufs=3) as xtp,          # x^T groups (sync q)
            tc.tile_pool(name="upool", bufs=2) as upool,      # exp-bias shear tiles
            tc.tile_pool(name="pexp", bufs=3) as pexpp,       # probs tiles
            tc.tile_pool(name="stage", bufs=2) as stage,      # normalize staging
            tc.tile_pool(name="outp", bufs=3) as outp,        # out staging
            tc.tile_pool(name="psum", bufs=4, space="PSUM") as psum,  # [128,1024] slots
            tc.tile_pool(name="dram", bufs=2, space="DRAM") as dramp,
        ):
            # ---------- constants / resident tensors ----------
            mask_sb = res.tile([P, NKT], DT.float32, tag="mask")
            # mask + ACT exp-table warm-up ride the (otherwise idle) scalar
            # queue so the sync/gpsimd queues start with the critical loads
            nc.scalar.dma_start(mask_sb[:], mask.ap().rearrange("(kt p) -> p kt", p=P))

            wq_sb = res.tile([P, NDT, HL * D_KV], DT.bfloat16, tag="wq")
            wk_sb = res.tile([P, NDT, HL * D_KV], DT.bfloat16, tag="wk")
            wv_sb = res.tile([P, NDT, HL * D_KV], DT.bfloat16, tag="wv")
            wo_sb = res.tile([P, NMT, D], DT.bfloat16, tag="wo")

            # persistent activations.  qt/kt/ctxt are split PER PAIR so the
            # tile dep tracker never serializes pair pr's score reads behind
            # pair pr+1's projection-drain writes (false WAR at qc bounds).
            qt_p = [res.tile([P, S], DT.bfloat16, tag=f"qt{m}", name=f"qt{m}")
                    for m in range(NMT)]                       # q REVERSED
            kt_p = [res.tile([P, S], DT.bfloat16, tag=f"kt{m}", name=f"kt{m}")
                    for m in range(NMT)]
            vaug = res.tile([P, NKT, NPAIR, VW], DT.bfloat16, tag="vaug")
            ctxt_p = [res.tile([P, S], DT.bfloat16, tag=f"ctxt{m}", name=f"ctxt{m}")
                      for m in range(NMT)]
            # only the two ones-columns are ever read outside the V blocks
            # (psum rows other than the denominator rows are never consumed)
            nc.vector.memset(vaug[:, :, :, 64:65], 1.0)
            nc.vector.memset(vaug[:, :, :, 97:98], 1.0)

            # ACT exp table warm-up (hide the ~2.7us table load under phase B)
            warm = res.tile([1, 2], DT.float32, tag="warm")
            nc.scalar.activation(out=warm[0:1, 0:1], in_=mask_sb[0:1, 0:1], func=AF.Exp)

            def rev_ap(base, jg0):
                """reversed-q view: base is a [rows, S] AP slice of a res
                tensor; returns [rows, SC] AP walking q backwards so writing
                reversed data lands in natural order."""
                return bass.AP(
                    tensor=base.tensor,
                    offset=base.offset + (S - 1 - jg0),
                    ap=[list(base.ap[0]), [-1, SC]],
                )

            UQ = W_U // 4
            def load_u(pr, quarter=None, u=None, eng=None):
                """erel shear tile [P, 2, W_U] for pair pr: u[p, i, w] =
                erel[2*pr+i, p + w].  quarter=None loads everything;
                otherwise loads one quarter of each head's span into the
                passed tile (spreads the ~2 MB burst across the previous
                pair's four q-chunks)."""
                if u is None:
                    u = upool.tile([P, 2, W_U], DT.bfloat16, tag="u",
                                   name=f"u{pr}", bufs=2)
                ap0 = erel.ap()
                qs = range(4) if quarter is None else [quarter]
                for i, hh in enumerate((2 * pr, 2 * pr + 1)):
                    for qq in qs:
                        shear = bass.AP(
                            tensor=ap0.tensor,
                            offset=ap0.offset + hh * NDIAG + qq * UQ,
                            ap=[[1, P], [1, UQ]],
                        )
                        (eng or nc.sync).dma_start(
                            u[:, i, qq * UQ:(qq + 1) * UQ], shear)
                return u

            def load_wchunk(g):
                """one 4-kd chunk of weights, all on gpsimd: the FIFO gives
                strict kd-order priority so the first matmuls never wait on
                later chunks or table loads."""
                cw = HL * D_KV
                c0, c1 = g * 4 * cw, (g + 1) * 4 * cw
                nc.gpsimd.dma_start(wq_sb[:, g * 4:(g + 1) * 4, :], wq[:, c0:c1])
                nc.gpsimd.dma_start(wk_sb[:, g * 4:(g + 1) * 4, :], wk[:, c0:c1])
                nc.gpsimd.dma_start(wv_sb[:, g * 4:(g + 1) * 4, :], wv[:, c0:c1])

            def load_xgroup(nq, g):
                """one [128, 4, 512] x^T kd-group (4KB partition lines)."""
                t = xtp.tile([P, 4, SC], DT.bfloat16, tag="xt",
                             name=f"xg{nq}_{g}")
                nc.sync.dma_start(t[:], xt[:, nq, 4 * g:4 * (g + 1), :])
                return t

            # ---------- phase B: pair-0 Q/K + V (all heads), single x^T pass ----
            # first weight chunk + first x group lead their queues so the
            # first matmul fires as soon as ~1MB lands (~9us)
            load_wchunk(0)
            for nq in range(NQC):
                qk_ps = psum.tile([P, 2 * SC], DT.float32, tag="ps",
                                  name=f"qkps0_{nq}")
                q_ps, k_ps = qk_ps[:, 0:SC], qk_ps[:, SC:2 * SC]
                v01 = psum.tile([P, 2 * SC], DT.float32, tag="ps", name=f"v01_{nq}")
                v23 = psum.tile([P, 2 * SC], DT.float32, tag="ps", name=f"v23_{nq}")
                v_ps = [v01[:, 0:SC], v01[:, SC:2 * SC],
                        v23[:, 0:SC], v23[:, SC:2 * SC]]
                for g in range(NKG):
                    if nq == 0 and g + 1 < NKG:
                        load_wchunk(g + 1)   # prefetch next weight chunk
                    xg = load_xgroup(nq, g)
                    for c in range(4):
                        kd = 4 * g + c
                        xt_t = xg[:, c, :]
                        nc.tensor.matmul(
                            q_ps, wq_sb[:, kd, 0:P], xt_t,
                            start=(kd == 0), stop=(kd == NDT - 1),
                        )
                        nc.tensor.matmul(
                            k_ps, wk_sb[:, kd, 0:P], xt_t,
                            start=(kd == 0), stop=(kd == NDT - 1),
                        )
                        for st in range(4):
                            nc.tensor.matmul(
                                v_ps[st], xg[:, c, st * P:(st + 1) * P],
                                wv_sb[:, kd, :],
                                start=(kd == 0), stop=(kd == NDT - 1),
                            )
                if nq == 0:
                    nc.scalar.dma_start(
                        wo_sb.rearrange("p a b -> p (a b)"), wo[:])
                # pair-0 u table: one quarter per nq, behind the weight
                # chunks on gpsimd (phase C only needs it ~90us in)
                u0 = load_u(0, quarter=nq, u=None if nq == 0 else u0,
                            eng=nc.gpsimd)
                # drain: V -> vaug blocks first (frees the 2 V psum slots the
                # next nq's V matmuls are waiting on), then q/k casts
                for st in range(4):
                    ktg = nq * 4 + st
                    vsrc = v_ps[st].rearrange("p (pr par d) -> p pr par d",
                                              par=2, d=D_KV)
                    nc.vector.tensor_copy(vaug[:, ktg, :, 0:D_KV],
                                          vsrc[:, :, 0, :])
                    nc.vector.tensor_copy(vaug[:, ktg, :, 129:193],
                                          vsrc[:, :, 1, :])
                nc.vector.tensor_copy(rev_ap(qt_p[0][:, :], nq * SC), q_ps)
                nc.vector.tensor_copy(kt_p[0][:, nq * SC:(nq + 1) * SC], k_ps)

            # ---------- phase C: attention, proj of pair pr+1 interleaved ----
            def emit_sg(pr, qc, kt):
                """scores psum group for (pair, q-chunk, k-tile): the two
                heads run as concurrent 64-row-group tiles."""
                jg0 = qc * SC
                s01 = psum.tile([P, 2 * SC], DT.float32, tag="ps",
                                name=f"s{pr}_{qc}_{kt}")
                nc.tensor.matmul(
                    s01[:, 0:SC], kt_p[pr][0:64, kt * P:(kt + 1) * P],
                    qt_p[pr][0:64, jg0:jg0 + SC],
                    start=True, stop=True, tile_position=(0, 0),
                )
                nc.tensor.matmul(
                    s01[:, SC:2 * SC], kt_p[pr][64:128, kt * P:(kt + 1) * P],
                    qt_p[pr][64:128, jg0:jg0 + SC],
                    start=True, stop=True, tile_position=(64, 0),
                )
                return s01

            # proj x^T group tiles, prefetched one group ahead (keyed by
            # group index within the current (proj, qc))
            def load_pgroup(proj, qc, g):
                t = xtp.tile([P, 4, SC], DT.bfloat16, tag="xt",
                             name=f"xp{proj}_{qc}_{g}")
                nc.sync.dma_start(t[:], xt[:, qc, 4 * g:4 * (g + 1), :])
                return t

            def attn_qc(pr, qc, u_t, proj, pending, s_pre, nxt_sg, pg0,
                        upf=None):
                """attention for head pair pr, reversed-q chunk qc.
                proj: None or pr+1 (emit that pair's Q/K proj, 1 kd per kt).
                pg0: pre-loaded x^T group 0 for the proj (or None).
                Returns (normalize closure, pre-emitted next score group,
                pre-loaded group 0 for the NEXT (proj, qc), next-pair u)."""
                u_ret = None
                h0, h1 = 2 * pr, 2 * pr + 1
                jg0 = qc * SC
                cx01 = psum.tile([P, 2 * SC], DT.float32, tag="ps",
                                 name=f"cx{pr}_{qc}")
                if proj is not None:
                    pj_ps = psum.tile([P, 2 * SC], DT.float32, tag="ps",
                                      name=f"pjps{proj}_{qc}")
                    pjq, pjk = pj_ps[:, 0:SC], pj_ps[:, SC:2 * SC]
                    pgs = {0: pg0}

                def emit_proj(kt):
                    g, c = kt // 4, kt % 4
                    if c == 0 and g + 1 < NKG:
                        pgs[g + 1] = load_pgroup(proj, qc, g + 1)
                    kd = kt
                    xt_t = pgs[g][:, c, :]
                    nc.tensor.matmul(
                        pjq, wq_sb[:, kd, proj * P:(proj + 1) * P], xt_t,
                        start=(kd == 0), stop=(kd == NDT - 1),
                    )
                    nc.tensor.matmul(
                        pjk, wk_sb[:, kd, proj * P:(proj + 1) * P], xt_t,
                        start=(kd == 0), stop=(kd == NDT - 1),
                    )

                # 2-deep software pipeline: s(kt+2) is emitted before PV(kt)
                # so the in-order PE queue keeps a backlog (hides LDWEIGHTS
                # and cross-engine semaphore latency).  pending() emits the
                # PREVIOUS qc's deferred normalize chain (DVE+DMA only).
                sq = [s_pre if s_pre is not None else emit_sg(pr, qc, 0),
                      emit_sg(pr, qc, 1)]
                for kt in range(NKT):
                    if kt + 2 < NKT:
                        sq.append(emit_sg(pr, qc, kt + 2))
                    if proj is not None:
                        emit_proj(kt)
                    if kt == 2 and pending is not None:
                        pending()
                    if kt == 8 and upf is not None:
                        u_ret = upf()
                    s01 = sq[kt]
                    px = pexpp.tile([P, 2 * SC], DT.bfloat16, tag="pexp",
                                    name=f"px{pr}_{qc}_{kt}")
                    nc.scalar.activation(
                        out=px[:], in_=s01[:], func=AF.Exp,
                        bias=mask_sb[:, kt:kt + 1], scale=1.0 / math.sqrt(D_KV),
                    )
                    j0 = kt * P + jg0
                    nc.vector.tensor_tensor(
                        px.rearrange("p (h j) -> p h j", h=2),
                        px.rearrange("p (h j) -> p h j", h=2),
                        u_t[:, :, j0:j0 + SC], OP.mult
                    )
                    nc.tensor.matmul(
                        cx01[0:65, 0:SC], vaug[:, kt, pr, 0:65], px[:, 0:SC],
                        start=(kt == 0), stop=(kt == NKT - 1),
                    )
                    nc.tensor.matmul(
                        cx01[:, SC:2 * SC], vaug[:, kt, pr, 65:VW],
                        px[:, SC:2 * SC],
                        start=(kt == 0), stop=(kt == NKT - 1),
                    )

                # proj drain (reversed q for qt)
                if proj is not None:
                    nc.scalar.copy(rev_ap(qt_p[proj][:, :], jg0), pjq)
                    nc.vector.tensor_copy(
                        kt_p[proj][:, jg0:jg0 + SC], pjk)

                # prefetch group 0 of the NEXT (proj, qc)'s x^T
                pg_next = None
                if qc + 1 < NQC and proj is not None:
                    pg_next = load_pgroup(proj, qc + 1, 0)
                elif qc == NQC - 1 and proj is not None and proj + 1 < NPAIR:
                    pg_next = load_pgroup(proj + 1, 0, 0)

                # pre-emit the NEXT qc's first score group so ACT never idles
                # across the boundary (exactly one PSUM slot is free here)
                s_next = nxt_sg() if nxt_sg is not None else None

                # ---- evacuate cx to SBUF (frees the PSUM slot), then the
                # rest of normalize+writeback is DEFERRED into the next qc
                # (DVE + DMA only; the PE never touches it) ----
                cxs = stage.tile([P, 2 * SC], DT.bfloat16, tag="cxs",
                                 name=f"cxs{pr}_{qc}", bufs=1)
                nc.vector.tensor_copy(cxs[:], cx01[:])

                def normalize():
                    # denominators: h0 on row 64 (cols 0:512), h1 on row 32
                    # (cols 512:1024).  Custom DVE ops need base-partition-0
                    # operands, so pack both rows into a base-0 tile first.
                    dnf = stage.tile([P, SC], DT.float32, tag="dnf",
                                     name=f"dnf{pr}_{qc}", bufs=1)
                    nc.vector.tensor_copy(dnf[64:65, :], cxs[64:65, 0:SC])
                    nc.vector.tensor_copy(dnf[32:33, :], cxs[32:33, SC:2 * SC])
                    rb = stage.tile([P, SC], DT.float32, tag="rb",
                                    name=f"rb{pr}_{qc}", bufs=1)
                    nc.vector.reciprocal_approx_fast(out=rb[:], in_=dnf[:])
                    rbh = stage.tile([P, SC], DT.bfloat16, tag="rbh",
                                     name=f"rbh{pr}_{qc}", bufs=1)
                    nc.vector.tensor_copy(rbh[64:65, :], rb[64:65, :])
                    nc.vector.tensor_copy(rbh[32:33, :], rb[32:33, :])
                    # broadcast across partitions: bounce the two reciprocal
                    # rows through DRAM, then stride-0 DMA reads replicate
                    # them to 64 partitions each (all off the engine queues).
                    bnc = dramp.tile([2, SC], DT.bfloat16, tag="bnc",
                                     name=f"bnc{pr}_{qc}")
                    nc.gpsimd.dma_start(bnc[0:1, :], rbh[64:65, :])
                    nc.gpsimd.dma_start(bnc[1:2, :], rbh[32:33, :])
                    bc_sb = stage.tile([P, SC], DT.bfloat16, tag="bc",
                                       name=f"bcs{pr}_{qc}", bufs=1)
                    src0 = bass.AP(tensor=bnc.tensor, offset=bnc.offset,
                                   ap=[[0, 64], [1, SC]])
                    src1 = bass.AP(tensor=bnc.tensor, offset=bnc.offset + SC,
                                   ap=[[0, 64], [1, SC]])
                    nc.gpsimd.dma_start(bc_sb[0:64, :], src0)
                    nc.gpsimd.dma_start(bc_sb[64:128, :], src1)
                    nc.vector.tensor_tensor(
                        rev_ap(ctxt_p[pr][0:64, :], jg0),
                        cxs[0:64, 0:SC], bc_sb[0:64, :], OP.mult)
                    nc.vector.tensor_tensor(
                        rev_ap(ctxt_p[pr][64:128, :], jg0),
                        cxs[64:128, SC:2 * SC], bc_sb[64:128, :], OP.mult)
                return normalize, s_next, pg_next, u_ret

            u_t = u0  # pair-0 table already quarter-loaded during phase B
            pending = None
            s_pre = None
            pg0 = load_pgroup(1, 0, 0)
            seq = [(pr, qc) for pr in range(NPAIR) for qc in range(NQC)]
            next_u = None
            for idx, (pr, qc) in enumerate(seq):
                nxt = pr + 1 if pr + 1 < NPAIR else None
                if nxt is not None:
                    upf = (lambda nxt=nxt, qc=qc, u=next_u:
                           load_u(nxt, quarter=qc, u=u, eng=nc.gpsimd))
                else:
                    upf = None
                if idx + 1 < len(seq):
                    npr, nqc = seq[idx + 1]
                    nxt_sg = (lambda npr=npr, nqc=nqc: emit_sg(npr, nqc, 0))
                else:
                    nxt_sg = None
                pending, s_pre, pg0, u_ret = attn_qc(pr, qc, u_t, nxt, pending,
                                                     s_pre, nxt_sg, pg0, upf)
                if u_ret is not None:
                    next_u = u_ret
                if qc == NQC - 1 and nxt is not None:
                    u_t = next_u
                    next_u = None
            pending()

            # ---------- phase D: output projection (descending st: the
            # low-st tiles depend on the last deferred normalize) ----------
            for st in range(NKT - 1, -1, -1):
                oa = psum.tile([P, 2 * SC], DT.float32, tag="ps",
                               name=f"oa{st}")
                ob = psum.tile([P, 2 * SC], DT.float32, tag="ps",
                               name=f"ob{st}")
                o_ps = [oa[:, 0:SC], oa[:, SC:2 * SC],
                        ob[:, 0:SC], ob[:, SC:2 * SC]]
                for m in range(NMT):
                    for nd in range(NQC):
                        nc.tensor.matmul(
                            o_ps[nd], ctxt_p[m][:, st * P:(st + 1) * P],
                            wo_sb[:, m, nd * SC:(nd + 1) * SC],
                            start=(m == 0), stop=(m == NMT - 1),
                        )
                for half in range(2):
                    o_t = outp.tile([P, 2, SC], DT.float32, tag="out",
                                    name=f"ot{st}_{half}")
                    nc.scalar.copy(o_t[:, 0, :], o_ps[2 * half])
                    nc.vector.tensor_copy(o_t[:, 1, :], o_ps[2 * half + 1])
                    eng = nc.sync if half == 0 else nc.gpsimd
                    eng.dma_start(
                        out[st * P:(st + 1) * P,
                            half * 2 * SC:(half + 1) * 2 * SC],
                        o_t[:],
                    )

    nc.finalize()
    return nc


_NC_CACHE = None


def _get_nc():
    global _NC_CACHE
    if _NC_CACHE is None:
        _NC_CACHE = _build()
    return _NC_CACHE


def _in_maps(hidden_states, attention_mask, Wq, Wk, Wv, Wo, rel_emb):
    import ml_dtypes
    bf16 = ml_dtypes.bfloat16
    maps = []
    for c in range(NCORES):
        b, g = c // 4, c % 4
        hlo, hhi = g * HL, (g + 1) * HL
        erel = _bias_table(
            np.ascontiguousarray(rel_emb[:, hlo:hhi], dtype=np.float32))
        def shuf(w):  # [NDT*P, C] -> [P, NDT*C] partition-contiguous
            cc = w.shape[1]
            return np.ascontiguousarray(
                w.reshape(-1, P, cc).transpose(1, 0, 2).reshape(P, -1))
        # xt[p, qc, kd, j] = x[qc*512+j, kd*128+p]
        xtt = np.ascontiguousarray(
            hidden_states[b].reshape(NQC, SC, NDT, P).transpose(3, 0, 2, 1)
        ).astype(bf16)
        maps.append({
            "xt": xtt,
            "wq": shuf(Wq[:, hlo * D_KV:hhi * D_KV]).astype(bf16),
            "wk": shuf(Wk[:, hlo * D_KV:hhi * D_KV]).astype(bf16),
            "wv": shuf(Wv[:, hlo * D_KV:hhi * D_KV]).astype(bf16),
            "wo": shuf(Wo[hlo * D_KV:hhi * D_KV, :]).astype(bf16),
            "mask": np.ascontiguousarray(attention_mask[b, 0, 0, :]).astype(np.float32),
            "erel": erel,
        })
    return maps


def kernel(hidden_states, attention_mask, Wq, Wk, Wv, Wo, rel_emb, _trace=False,
           _trace_kwargs=None):
    hidden_states = np.asarray(hidden_states, dtype=np.float32)
    attention_mask = np.asarray(attention_mask, dtype=np.float32)
    Wq = np.asarray(Wq, dtype=np.float32)
    Wk = np.asarray(Wk, dtype=np.float32)
    Wv = np.asarray(Wv, dtype=np.float32)
    Wo = np.asarray(Wo, dtype=np.float32)
    rel_emb = np.asarray(rel_emb, dtype=np.float32)

    nc = _get_nc()
    maps = _in_maps(hidden_states, attention_mask, Wq, Wk, Wv, Wo, rel_emb)
    kw = dict(_trace_kwargs or {})
    res = run_bass_kernel_spmd(nc, maps, core_ids=list(range(NCORES)),
                               trace=_trace, **kw)
    kernel.last_results = res
    outp = np.empty((B, S, D), dtype=np.float32)
    for b in range(B):
        acc = np.asarray(res.results[4 * b]["out"], dtype=np.float32).copy()
        for g in range(1, 4):
            acc += np.asarray(res.results[4 * b + g]["out"], dtype=np.float32)
        outp[b] = acc
    return outp


# revision 14
# speedup vs baseline: 1.0731x; 1.0224x over previous
"""T5-style encoder self-attention (dense_transformer) on 8 Trainium2 NeuronCores.

Problem (full shapes): hidden [2,2048,2048], Wq/Wk/Wv/Wo [2048,2048],
rel_emb [32,32] (bidirectional T5 relative-position bias), mask [2,1,1,2048].

Sharding: data-parallel over batch (2) x tensor-parallel over heads (4 groups
of 8 heads) = 8 cores, Megatron-style. Each core computes a partial output
[2048,2048] for its batch (its 8 heads through its Wo row-slice); the host
sums 4 partials per batch.

Per-core kernel design (bf16 operands, fp32 PSUM accumulation):
  - The relative-position bias is applied MULTIPLICATIVELY for every tile:
    px = exp(s/8 + mask) * erel, where erel = exp(bias) is a host-computed
    [8 heads, 4096 diagonals] bf16 table read through a Toeplitz shear view
    (partition stride 1, free stride 1).  Host numerics sim puts the
    all-multiplicative absmax-rel at 7.7e-3 (vs 6.0e-3 for the old scheme
    that identity-injected near-diagonal tiles into PSUM; the inject matmuls
    cost ~38us of PE issue time and a 16us identity DMA on the critical
    startup path).
  - Phase B: single pass over x^T computes pair-0 Q^T/K^T and V for ALL
    heads (6 matmuls per x^T chunk, PE-bound).  Q^T is stored with s
    REVERSED so the bias becomes a positive-shear Toeplitz.
  - Phase C attention, per (head-pair, q-chunk), k-tile loop pipelined one
    iteration ahead:
      * the two per-head QK matmuls are packed as concurrent 64-row-group
        tiles (tile_position (0,0)/(64,0));
      * ACT computes exp(s/8 + mask) in one [128,1024] shot per k-tile;
        DVE multiplies by the erel shear slice (far AND near tiles);
      * next-pair Q/K projection matmuls are interleaved PER k-tile so they
        fill the PE's ACT-wait bubbles; their x^T tiles are group-loaded
        (4 k-chunks per DMA, 4KB partition lines) and prefetched one group
        ahead so the proj matmuls never wait on DMA.
  - x^T is host-tiled to [128, NQC, NDT, 512] so every [128, 4, 512] group
    load has 4KB contiguous per-partition lines (the old [D,S] layout gave
    1KB lines, which capped each DMA queue near 85 GB/s and stalled the
    interleaved projections).
  - V augmentation: per pair, even head block = [v(0:64) | ones(64)] (M=65,
    denominator lands on PSUM partition 64), odd head block = 128 wide with
    ones at col 32 and v at cols 64:128 (denominator on partition 32, ctx on
    partitions 64:128), keeping every normalize op partition-aligned.
  - Normalize is DEFERRED and PE-free: cx evacuates to SBUF at qc end
    (freeing its PSUM slot), then one qc later a DVE+DMA-only chain runs:
    pack denominator rows to a base-0 tile (custom DVE ops require base
    partition 0), reciprocal_approx_fast, bounce the two reciprocal rows
    through DRAM, stride-0 DMAs broadcast them across partitions, and fused
    DVE tensor_tensors do normalize + un-reverse + bf16 writeback.
  - The next qc's first score-group is pre-emitted in the current qc's tail
    (exactly one PSUM slot is free there) so ACT never idles at boundaries.
  - Startup: the first x^T group and the first wq/wk/wv chunk are the FIRST
    DMAs on their queues (sync / gpsimd), so the first matmul fires ~9us in
    instead of ~41us; mask + ACT-table warmup + wo ride the scalar queue.
  - Phase D output projection: descending s-tiles (low tiles depend on the
    last deferred normalize), m looped inside nd so consecutive matmuls hit
    different PSUM banks; evacuation alternates ACT/DVE; the two out DMAs
    per s-tile alternate sync/gpsimd queues.
"""

import math
import sys

for _p in ("/opt/trn_rl_repo",):
    if _p not in sys.path:
        sys.path.insert(0, _p)

import numpy as np

import concourse.bass as bass
import concourse.mybir as mybir
import concourse.tile as tile
from concourse import bacc
from concourse.bass_utils import run_bass_kernel_spmd

DT = mybir.dt
AF = mybir.ActivationFunctionType
OP = mybir.AluOpType

# ---- problem constants (hardcoded per contract) ----
B, S, D = 2, 2048, 2048
N_HEADS, D_KV = 32, 64
NUM_BUCKETS, MAX_DISTANCE = 32, 128
NCORES = 8
HL = 8            # heads per core
P = 128
SC = 512          # free-dim chunk
NKT = S // P      # 16 k-tiles
NQC = S // SC     # 4 q-chunks
NDT = D // P      # 16 D-tiles
NMT = (HL * D_KV) // P   # 4 hd m-tiles per core
NPAIR = HL // 2   # 4 head pairs per core
NDIAG = 4096
W_U = 3968        # erel shear tile width (covers all diagonals any tile hits)
VW = 193          # vaug per-(kt,pair) width: even block 65 + odd block 128
NKG = NDT // 4    # 4 kd-groups of 4 chunks per q-chunk (x^T group loads)


def _rel_bucket_host(d):
    """Exact numpy replica of reference._relative_position_bucket."""
    num_buckets = NUM_BUCKETS // 2          # 16
    max_exact = num_buckets // 2            # 8
    rel = np.asarray(d, dtype=np.int64)
    buckets = (rel > 0).astype(np.int32) * num_buckets
    arel = np.abs(rel)
    is_small = arel < max_exact
    rp_safe = np.maximum(arel, 1).astype(np.float32)
    log_ratio = np.log(rp_safe / np.float32(max_exact)).astype(np.float32)
    scale = np.float32(math.log(MAX_DISTANCE / max_exact))
    rp_large = max_exact + (log_ratio / scale * np.float32(num_buckets - max_exact)).astype(np.int32)
    rp_large = np.minimum(rp_large, num_buckets - 1)
    buckets = buckets + np.where(is_small, arel.astype(np.int32), rp_large)
    return buckets.astype(np.int32)


def _bias_table(rel_emb_slice):
    """rel_emb_slice: [NUM_BUCKETS, HL] fp32 -> erel [HL, NDIAG] bf16,
    erel[h, i] = exp(bias(d = i - 2047)); erel[:, 4095] is never read."""
    import ml_dtypes
    i = np.arange(NDIAG - 1)
    b = _rel_bucket_host(i - (S - 1))                  # [4095]
    vals = rel_emb_slice[b, :]                         # [4095, HL] fp32
    erel = np.zeros((HL, NDIAG), dtype=np.float32)
    erel[:, : NDIAG - 1] = np.exp(vals.T)
    return erel.astype(ml_dtypes.bfloat16)


def _build():
    nc = bacc.Bacc(None, name="attn_tp")

    # x^T host-tiled: xt[p, qc, kd, j] = x[qc*512+j, kd*128+p], so a
    # [128, 4, 512] kd-group load is one DMA with 4KB per-partition lines
    xt = nc.declare_dram_parameter("xt", [P, NQC, NDT, SC], DT.bfloat16,
                                   isOutput=False)
    # weights arrive HOST-SHUFFLED to [p][kt][h] so per-partition lines are
    # contiguous multi-KB runs (DMA packet rate is the limiter at 1KB lines)
    wq = nc.declare_dram_parameter("wq", [P, NDT * HL * D_KV], DT.bfloat16, isOutput=False)
    wk = nc.declare_dram_parameter("wk", [P, NDT * HL * D_KV], DT.bfloat16, isOutput=False)
    wv = nc.declare_dram_parameter("wv", [P, NDT * HL * D_KV], DT.bfloat16, isOutput=False)
    wo = nc.declare_dram_parameter("wo", [P, NMT * D], DT.bfloat16, isOutput=False)
    mask = nc.declare_dram_parameter("mask", [S], DT.float32, isOutput=False)
    erel = nc.declare_dram_parameter("erel", [HL, NDIAG], DT.bfloat16, isOutput=False)
    out = nc.declare_dram_parameter("out", [S, D], DT.float32, isOutput=True)

    with tile.TileContext(nc) as tc:
        with (
            tc.tile_pool(name="res", bufs=1) as res,          # persistent tensors
            tc.tile_pool(name="xtp", bufs=3) as xtp,          # x^T groups (sync q)
            tc.tile_pool(name="upool", bufs=2) as upool,      # exp-bias shear tiles
            tc.tile_pool(name="pexp", bufs=3) as pexpp,       # probs tiles
            tc.tile_pool(name="stage", bufs=2) as stage,      # normalize staging
            tc.tile_pool(name="outp", bufs=3) as outp,        # out staging
            tc.tile_pool(name="psum", bufs=4, space="PSUM") as psum,  # [128,1024] slots
            tc.tile_pool(name="dram", bufs=2, space="DRAM") as dramp,
        ):
            # ---------- constants / resident tensors ----------
            mask_sb = res.tile([P, NKT], DT.float32, tag="mask")
            # mask + ACT exp-table warm-up ride the (otherwise idle) scalar
            # queue so the sync/gpsimd queues start with the critical loads
            nc.scalar.dma_start(mask_sb[:], mask.ap().rearrange("(kt p) -> p kt", p=P))

            wq_sb = res.tile([P, NDT, HL * D_KV], DT.bfloat16, tag="wq")
            wk_sb = res.tile([P, NDT, HL * D_KV], DT.bfloat16, tag="wk")
            wv_sb = res.tile([P, NDT, HL * D_KV], DT.bfloat16, tag="wv")
            wo_sb = res.tile([P, NMT, D], DT.bfloat16, tag="wo")

            # persistent activations.  qt/kt/ctxt are split PER PAIR so the
            # tile dep tracker never serializes pair pr's score reads behind
            # pair pr+1's projection-drain writes (false WAR at qc bounds).
            qt_p = [res.tile([P, S], DT.bfloat16, tag=f"qt{m}", name=f"qt{m}")
                    for m in range(NMT)]                       # q REVERSED
            kt_p = [res.tile([P, S], DT.bfloat16, tag=f"kt{m}", name=f"kt{m}")
                    for m in range(NMT)]
            vaug = res.tile([P, NKT, NPAIR, VW], DT.bfloat16, tag="vaug")
            ctxt_p = [res.tile([P, S], DT.bfloat16, tag=f"ctxt{m}", name=f"ctxt{m}")
                      for m in range(NMT)]
            # only the two ones-columns are ever read outside the V blocks
            # (psum rows other than the denominator rows are never consumed)
            nc.vector.memset(vaug[:, :, :, 64:65], 1.0)
            nc.vector.memset(vaug[:, :, :, 97:98], 1.0)

            # ACT exp table warm-up (hide the ~2.7us table load under phase B)
            warm = res.tile([1, 2], DT.float32, tag="warm")
            nc.scalar.activation(out=warm[0:1, 0:1], in_=mask_sb[0:1, 0:1], func=AF.Exp)

            def rev_ap(base, jg0):
                """reversed-q view: base is a [rows, S] AP slice of a res
                tensor; returns [rows, SC] AP walking q backwards so writing
                reversed data lands in natural order."""
                return bass.AP(
                    tensor=base.tensor,
                    offset=base.offset + (S - 1 - jg0),
                    ap=[list(base.ap[0]), [-1, SC]],
                )

            UQ = W_U // 4
            def load_u(pr, quarter=None, u=None, eng=None):
                """erel shear tile [P, 2, W_U] for pair pr: u[p, i, w] =
                erel[2*pr+i, p + w].  quarter=None loads everything;
                otherwise loads one quarter of each head's span into the
                passed tile (spreads the ~2 MB burst across the previous
                pair's four q-chunks)."""
                if u is None:
                    u = upool.tile([P, 2, W_U], DT.bfloat16, tag="u",
                                   name=f"u{pr}", bufs=2)
                ap0 = erel.ap()
                qs = range(4) if quarter is None else [quarter]
                for i, hh in enumerate((2 * pr, 2 * pr + 1)):
                    for qq in qs:
                        shear = bass.AP(
                            tensor=ap0.tensor,
                            offset=ap0.offset + hh * NDIAG + qq * UQ,
                            ap=[[1, P], [1, UQ]],
                        )
                        (eng or nc.sync).dma_start(
                            u[:, i, qq * UQ:(qq + 1) * UQ], shear)
                return u

            def load_wchunk(g, eng=None):
                """one 4-kd chunk of weights; chunk 0 rides sync (HWDGE,
                ~0.6us first byte) right behind the first x group so the
                first matmuls fire ~10us in; later chunks stream on gpsimd
                in kd order."""
                eng = eng or nc.gpsimd
                cw = HL * D_KV
                c0, c1 = g * 4 * cw, (g + 1) * 4 * cw
                eng.dma_start(wq_sb[:, g * 4:(g + 1) * 4, :], wq[:, c0:c1])
                eng.dma_start(wk_sb[:, g * 4:(g + 1) * 4, :], wk[:, c0:c1])
                eng.dma_start(wv_sb[:, g * 4:(g + 1) * 4, :], wv[:, c0:c1])

            def load_xgroup(nq, g):
                """one [128, 4, 512] x^T kd-group (4KB partition lines)."""
                t = xtp.tile([P, 4, SC], DT.bfloat16, tag="xt",
                             name=f"xg{nq}_{g}")
                nc.sync.dma_start(t[:], xt[:, nq, 4 * g:4 * (g + 1), :])
                return t

            # ---------- phase B: pair-0 Q/K + V (all heads), single x^T pass ----
            wc0_loaded = False
            for nq in range(NQC):
                qk_ps = psum.tile([P, 2 * SC], DT.float32, tag="ps",
                                  name=f"qkps0_{nq}")
                q_ps, k_ps = qk_ps[:, 0:SC], qk_ps[:, SC:2 * SC]
                v01 = psum.tile([P, 2 * SC], DT.float32, tag="ps", name=f"v01_{nq}")
                v23 = psum.tile([P, 2 * SC], DT.float32, tag="ps", name=f"v23_{nq}")
                v_ps = [v01[:, 0:SC], v01[:, SC:2 * SC],
                        v23[:, 0:SC], v23[:, SC:2 * SC]]
                for g in range(NKG):
                    xg = load_xgroup(nq, g)
                    if nq == 0 and not wc0_loaded:
                        load_wchunk(0, eng=nc.sync)
                        wc0_loaded = True
                    if nq == 0 and g + 1 < NKG:
                        load_wchunk(g + 1)   # prefetch next weight chunk
                    for c in range(4):
                        kd = 4 * g + c
                        xt_t = xg[:, c, :]
                        nc.tensor.matmul(
                            q_ps, wq_sb[:, kd, 0:P], xt_t,
                            start=(kd == 0), stop=(kd == NDT - 1),
                        )
                        nc.tensor.matmul(
                            k_ps, wk_sb[:, kd, 0:P], xt_t,
                            start=(kd == 0), stop=(kd == NDT - 1),
                        )
                        for st in range(4):
                            nc.tensor.matmul(
                                v_ps[st], xg[:, c, st * P:(st + 1) * P],
                                wv_sb[:, kd, :],
                                start=(kd == 0), stop=(kd == NDT - 1),
                            )
                if nq == 0:
                    nc.scalar.dma_start(
                        wo_sb.rearrange("p a b -> p (a b)"), wo[:])
                # pair-0 u table: one quarter per nq, behind the weight
                # chunks on gpsimd (phase C only needs it ~90us in)
                u0 = load_u(0, quarter=nq, u=None if nq == 0 else u0,
                            eng=nc.gpsimd)
                # drain: V -> vaug blocks first (frees the 2 V psum slots the
                # next nq's V matmuls are waiting on), then q/k casts
                for st in range(4):
                    ktg = nq * 4 + st
                    vsrc = v_ps[st].rearrange("p (pr par d) -> p pr par d",
                                              par=2, d=D_KV)
                    nc.vector.tensor_copy(vaug[:, ktg, :, 0:D_KV],
                                          vsrc[:, :, 0, :])
                    nc.vector.tensor_copy(vaug[:, ktg, :, 129:193],
                                          vsrc[:, :, 1, :])
                nc.vector.tensor_copy(rev_ap(qt_p[0][:, :], nq * SC), q_ps)
                nc.vector.tensor_copy(kt_p[0][:, nq * SC:(nq + 1) * SC], k_ps)

            # ---------- phase C: attention, proj of pair pr+1 interleaved ----
            def emit_sg(pr, qc, kt):
                """scores psum group for (pair, q-chunk, k-tile): the two
                heads run as concurrent 64-row-group tiles."""
                jg0 = qc * SC
                s01 = psum.tile([P, 2 * SC], DT.float32, tag="ps",
                                name=f"s{pr}_{qc}_{kt}")
                nc.tensor.matmul(
                    s01[:, 0:SC], kt_p[pr][0:64, kt * P:(kt + 1) * P],
                    qt_p[pr][0:64, jg0:jg0 + SC],
                    start=True, stop=True, tile_position=(0, 0),
                )
                nc.tensor.matmul(
                    s01[:, SC:2 * SC], kt_p[pr][64:128, kt * P:(kt + 1) * P],
                    qt_p[pr][64:128, jg0:jg0 + SC],
                    start=True, stop=True, tile_position=(64, 0),
                )
                return s01

            # proj x^T group tiles, prefetched one group ahead (keyed by
            # group index within the current (proj, qc))
            def load_pgroup(proj, qc, g):
                t = xtp.tile([P, 4, SC], DT.bfloat16, tag="xt",
                             name=f"xp{proj}_{qc}_{g}")
                nc.sync.dma_start(t[:], xt[:, qc, 4 * g:4 * (g + 1), :])
                return t

            def attn_qc(pr, qc, u_t, proj, pending, s_pre, nxt_sg, pg0,
                        upf=None):
                """attention for head pair pr, reversed-q chunk qc.
                proj: None or pr+1 (emit that pair's Q/K proj, 1 kd per kt).
                pg0: pre-loaded x^T group 0 for the proj (or None).
                Returns (normalize closure, pre-emitted next score group,
                pre-loaded group 0 for the NEXT (proj, qc), next-pair u)."""
                u_ret = None
                h0, h1 = 2 * pr, 2 * pr + 1
                jg0 = qc * SC
                cx01 = psum.tile([P, 2 * SC], DT.float32, tag="ps",
                                 name=f"cx{pr}_{qc}")
                if proj is not None:
                    pj_ps = psum.tile([P, 2 * SC], DT.float32, tag="ps",
                                      name=f"pjps{proj}_{qc}")
                    pjq, pjk = pj_ps[:, 0:SC], pj_ps[:, SC:2 * SC]
                    pgs = {0: pg0}

                def emit_proj(kt):
                    g, c = kt // 4, kt % 4
                    if c == 0 and g + 1 < NKG:
                        pgs[g + 1] = load_pgroup(proj, qc, g + 1)
                    kd = kt
                    xt_t = pgs[g][:, c, :]
                    nc.tensor.matmul(
                        pjq, wq_sb[:, kd, proj * P:(proj + 1) * P], xt_t,
                        start=(kd == 0), stop=(kd == NDT - 1),
                    )
                    nc.tensor.matmul(
                        pjk, wk_sb[:, kd, proj * P:(proj + 1) * P], xt_t,
                        start=(kd == 0), stop=(kd == NDT - 1),
                    )

                # 2-deep software pipeline: s(kt+2) is emitted before PV(kt)
                # so the in-order PE queue keeps a backlog (hides LDWEIGHTS
                # and cross-engine semaphore latency).  pending() emits the
                # PREVIOUS qc's deferred normalize chain (DVE+DMA only).
                sq = [s_pre if s_pre is not None else emit_sg(pr, qc, 0),
                      emit_sg(pr, qc, 1)]
                for kt in range(NKT):
                    if kt + 2 < NKT:
                        sq.append(emit_sg(pr, qc, kt + 2))
                    if proj is not None:
                        emit_proj(kt)
                    if kt == 2 and pending is not None:
                        pending[0]()
                    if kt == 8:
                        if pending is not None:
                            pending[1]()
                        if upf is not None:
                            u_ret = upf()
                    s01 = sq[kt]
                    px = pexpp.tile([P, 2 * SC], DT.bfloat16, tag="pexp",
                                    name=f"px{pr}_{qc}_{kt}")
                    nc.scalar.activation(
                        out=px[:], in_=s01[:], func=AF.Exp,
                        bias=mask_sb[:, kt:kt + 1], scale=1.0 / math.sqrt(D_KV),
                    )
                    j0 = kt * P + jg0
                    nc.vector.tensor_tensor(
                        px.rearrange("p (h j) -> p h j", h=2),
                        px.rearrange("p (h j) -> p h j", h=2),
                        u_t[:, :, j0:j0 + SC], OP.mult
                    )
                    nc.tensor.matmul(
                        cx01[0:65, 0:SC], vaug[:, kt, pr, 0:65], px[:, 0:SC],
                        start=(kt == 0), stop=(kt == NKT - 1),
                    )
                    nc.tensor.matmul(
                        cx01[:, SC:2 * SC], vaug[:, kt, pr, 65:VW],
                        px[:, SC:2 * SC],
                        start=(kt == 0), stop=(kt == NKT - 1),
                    )

                # proj drain (reversed q for qt)
                if proj is not None:
                    nc.scalar.copy(rev_ap(qt_p[proj][:, :], jg0), pjq)
                    nc.vector.tensor_copy(
                        kt_p[proj][:, jg0:jg0 + SC], pjk)

                # prefetch group 0 of the NEXT (proj, qc)'s x^T
                pg_next = None
                if qc + 1 < NQC and proj is not None:
                    pg_next = load_pgroup(proj, qc + 1, 0)
                elif qc == NQC - 1 and proj is not None and proj + 1 < NPAIR:
                    pg_next = load_pgroup(proj + 1, 0, 0)

                # pre-emit the NEXT qc's first score group so ACT never idles
                # across the boundary (exactly one PSUM slot is free here)
                s_next = nxt_sg() if nxt_sg is not None else None

                # ---- evacuate cx to SBUF (frees the PSUM slot), then the
                # rest of normalize+writeback is DEFERRED into the next qc
                # (DVE + DMA only; the PE never touches it) ----
                cxs = stage.tile([P, 2 * SC], DT.bfloat16, tag="cxs",
                                 name=f"cxs{pr}_{qc}", bufs=1)
                nc.vector.tensor_copy(cxs[:], cx01[:])

                bc_box = {}

                def norm_a():
                    # denominators: h0 on row 64 (cols 0:512), h1 on row 32
                    # (cols 512:1024).  Custom DVE ops need base-partition-0
                    # operands, so pack both rows into a base-0 tile first.
                    # Runs at kt==2 of the NEXT qc; the DRAM-bounce broadcast
                    # DMAs get ~8 k-tiles of latency slack before norm_b's
                    # tensor_tensors (at kt==8) consume bc_sb - the ~5us
                    # chain latency never blocks the Vector queue.
                    dnf = stage.tile([P, SC], DT.float32, tag="dnf",
                                     name=f"dnf{pr}_{qc}", bufs=1)
                    nc.vector.tensor_copy(dnf[64:65, :], cxs[64:65, 0:SC])
                    nc.vector.tensor_copy(dnf[32:33, :], cxs[32:33, SC:2 * SC])
                    rb = stage.tile([P, SC], DT.float32, tag="rb",
                                    name=f"rb{pr}_{qc}", bufs=1)
                    nc.vector.reciprocal_approx_fast(out=rb[:], in_=dnf[:])
                    rbh = stage.tile([P, SC], DT.bfloat16, tag="rbh",
                                     name=f"rbh{pr}_{qc}", bufs=1)
                    nc.vector.tensor_copy(rbh[64:65, :], rb[64:65, :])
                    nc.vector.tensor_copy(rbh[32:33, :], rb[32:33, :])
                    bnc = dramp.tile([2, SC], DT.bfloat16, tag="bnc",
                                     name=f"bnc{pr}_{qc}")
                    nc.gpsimd.dma_start(bnc[0:1, :], rbh[64:65, :])
                    nc.gpsimd.dma_start(bnc[1:2, :], rbh[32:33, :])
                    bc_sb = stage.tile([P, SC], DT.bfloat16, tag="bc",
                                       name=f"bcs{pr}_{qc}", bufs=1)
                    src0 = bass.AP(tensor=bnc.tensor, offset=bnc.offset,
                                   ap=[[0, 64], [1, SC]])
                    src1 = bass.AP(tensor=bnc.tensor, offset=bnc.offset + SC,
                                   ap=[[0, 64], [1, SC]])
                    nc.gpsimd.dma_start(bc_sb[0:64, :], src0)
                    nc.gpsimd.dma_start(bc_sb[64:128, :], src1)
                    bc_box["bc"] = bc_sb

                def norm_b():
                    bc_sb = bc_box["bc"]
                    nc.vector.tensor_tensor(
                        rev_ap(ctxt_p[pr][0:64, :], jg0),
                        cxs[0:64, 0:SC], bc_sb[0:64, :], OP.mult)
                    nc.vector.tensor_tensor(
                        rev_ap(ctxt_p[pr][64:128, :], jg0),
                        cxs[64:128, SC:2 * SC], bc_sb[64:128, :], OP.mult)
                return (norm_a, norm_b), s_next, pg_next, u_ret

            u_t = u0  # pair-0 table already quarter-loaded during phase B
            pending = None
            s_pre = None
            pg0 = load_pgroup(1, 0, 0)
            seq = [(pr, qc) for pr in range(NPAIR) for qc in range(NQC)]
            next_u = None
            for idx, (pr, qc) in enumerate(seq):
                nxt = pr + 1 if pr + 1 < NPAIR else None
                if nxt is not None:
                    upf = (lambda nxt=nxt, qc=qc, u=next_u:
                           load_u(nxt, quarter=qc, u=u, eng=nc.gpsimd))
                else:
                    upf = None
                if idx + 1 < len(seq):
                    npr, nqc = seq[idx + 1]
                    nxt_sg = (lambda npr=npr, nqc=nqc: emit_sg(npr, nqc, 0))
                else:
                    nxt_sg = None
                pending, s_pre, pg0, u_ret = attn_qc(pr, qc, u_t, nxt, pending,
                                                     s_pre, nxt_sg, pg0, upf)
                if u_ret is not None:
                    next_u = u_ret
                if qc == NQC - 1 and nxt is not None:
                    u_t = next_u
                    next_u = None
            pending[0]()
            pending[1]()

            # ---------- phase D: output projection (descending st: the
            # low-st tiles depend on the last deferred normalize) ----------
            for st in range(NKT - 1, -1, -1):
                oa = psum.tile([P, 2 * SC], DT.float32, tag="ps",
                               name=f"oa{st}")
                ob = psum.tile([P, 2 * SC], DT.float32, tag="ps",
                               name=f"ob{st}")
                o_ps = [oa[:, 0:SC], oa[:, SC:2 * SC],
                        ob[:, 0:SC], ob[:, SC:2 * SC]]
                for m in range(NMT):
                    for nd in range(NQC):
                        nc.tensor.matmul(
                            o_ps[nd], ctxt_p[m][:, st * P:(st + 1) * P],
                            wo_sb[:, m, nd * SC:(nd + 1) * SC],
                            start=(m == 0), stop=(m == NMT - 1),
                        )
                for half in range(2):
                    o_t = outp.tile([P, 2, SC], DT.float32, tag="out",
                                    name=f"ot{st}_{half}")
                    nc.scalar.copy(o_t[:, 0, :], o_ps[2 * half])
                    nc.vector.tensor_copy(o_t[:, 1, :], o_ps[2 * half + 1])
                    eng = nc.sync if half == 0 else nc.gpsimd
                    eng.dma_start(
                        out[st * P:(st + 1) * P,
                            half * 2 * SC:(half + 1) * 2 * SC],
                        o_t[:],
                    )

    nc.finalize()
    return nc


_NC_CACHE = None


def _get_nc():
    global _NC_CACHE
    if _NC_CACHE is None:
        _NC_CACHE = _build()
    return _NC_CACHE


def _in_maps(hidden_states, attention_mask, Wq, Wk, Wv, Wo, rel_emb):
    import ml_dtypes
    bf16 = ml_dtypes.bfloat16
    maps = []
    for c in range(NCORES):
        b, g = c // 4, c % 4
        hlo, hhi = g * HL, (g + 1) * HL
        erel = _bias_table(
            np.ascontiguousarray(rel_emb[:, hlo:hhi], dtype=np.float32))
        def shuf(w):  # [NDT*P, C] -> [P, NDT*C] partition-contiguous
            cc = w.shape[1]
            return np.ascontiguousarray(
                w.reshape(-1, P, cc).transpose(1, 0, 2).reshape(P, -1))
        # xt[p, qc, kd, j] = x[qc*512+j, kd*128+p]
        xtt = np.ascontiguousarray(
            hidden_states[b].reshape(NQC, SC, NDT, P).transpose(3, 0, 2, 1)
        ).astype(bf16)
        maps.append({
            "xt": xtt,
            "wq": shuf(Wq[:, hlo * D_KV:hhi * D_KV]).astype(bf16),
            "wk": shuf(Wk[:, hlo * D_KV:hhi * D_KV]).astype(bf16),
            "wv": shuf(Wv[:, hlo * D_KV:hhi * D_KV]).astype(bf16),
            "wo": shuf(Wo[hlo * D_KV:hhi * D_KV, :]).astype(bf16),
            "mask": np.ascontiguousarray(attention_mask[b, 0, 0, :]).astype(np.float32),
            "erel": erel,
        })
    return maps


def kernel(hidden_states, attention_mask, Wq, Wk, Wv, Wo, rel_emb, _trace=False,
           _trace_kwargs=None):
    hidden_states = np.asarray(hidden_states, dtype=np.float32)
    attention_mask = np.asarray(attention_mask, dtype=np.float32)
    Wq = np.asarray(Wq, dtype=np.float32)
    Wk = np.asarray(Wk, dtype=np.float32)
    Wv = np.asarray(Wv, dtype=np.float32)
    Wo = np.asarray(Wo, dtype=np.float32)
    rel_emb = np.asarray(rel_emb, dtype=np.float32)

    nc = _get_nc()
    maps = _in_maps(hidden_states, attention_mask, Wq, Wk, Wv, Wo, rel_emb)
    kw = dict(_trace_kwargs or {})
    res = run_bass_kernel_spmd(nc, maps, core_ids=list(range(NCORES)),
                               trace=_trace, **kw)
    kernel.last_results = res
    outp = np.empty((B, S, D), dtype=np.float32)
    for b in range(B):
        acc = np.asarray(res.results[4 * b]["out"], dtype=np.float32).copy()
        for g in range(1, 4):
            acc += np.asarray(res.results[4 * b + g]["out"], dtype=np.float32)
        outp[b] = acc
    return outp
